# revision 16
# baseline (speedup 1.0000x reference)
"""NormLinearAttention Trainium2 kernel — 8-core sequence-parallel Bass/Tile.

Math (reference):
    q = k = elu(heads(x @ Wqk + bqk));  v = heads(silu(x @ Wv + bv))
    u = silu(x @ Wu + bu)
    kv[b,h] = k^T v  (contract over sequence);  att = q @ kv
    y = (u * layernorm(att)) @ Wo + bo

Sharding: each of 8 cores owns 512 tokens of each batch (2048 tokens total).
Per-core partial kv is AllReduce-summed across cores in 4 chunks (1 MB each,
issued per head-quarter as phase 1 produces them, overlapped with the
u-projection); everything else is local.

v2 layout/schedule notes (changes vs v1):
  - q is spilled to DRAM token-major and re-loaded in phase 3 with the XBAR
    transposing DMA (dma_start_transpose), killing 256 PE transposes plus
    their PSUM/DVE copies.
  - v's silu is computed as 0.5*x*(1+tanh(x/2)) so all of phase 1 uses the
    exp activation-table set (elu needs Exp); avoids ~60 ACT_TABLE_LOADs.
  - rstd = exp(-0.5*ln(var+eps)) (natural_log_exp set; Rsqrt is banned and
    Sqrt+reciprocal costs a serial 3.3us DVE reciprocal per batch).
  - LN scalars are partition-broadcast on GpSimd (SBUF->SBUF), not via PE
    ones-matmuls into PSUM: frees 2 PSUM banks and avoids PE-FIFO stalls.
  - kv AllReduce chunked by head-quarter; phase 3 att starts as chunks land.
  - u/w weight loads + uT spill + y stores ride the scalar DMA queue so the
    sync queue keeps phase-critical traffic only; phase-1 issue order puts
    wq/wv ahead of the 8 MB xT load so the PE starts ~8us in, not 61us.
  - y is written bf16 (host upcasts); halves the output-DMA tail.
All matmuls bf16 (fp32 PSUM accumulate); elementwise/LN math fp32.
"""

import sys

if "/opt/trn_rl_repo" not in sys.path:
    sys.path.insert(0, "/opt/trn_rl_repo")

import numpy as np
import ml_dtypes

B, N, E = 4, 4096, 2048
H_DIM, HEADS, DH = 2048, 16, 128
N_CORES = 8
NL = N // N_CORES          # 512 tokens per (core, batch)
TL = B * NL                # 2048 local tokens per core
ET = E // 128              # 16 contraction tiles
JT = H_DIM // 128          # 16 hidden tiles
NJQ = 4                    # process hidden dim in 4 quarters of 512
TOK_B = NL // 128          # 4 token tiles per batch
LN_EPS = 1e-5

_BUILT = {}


def _build(flags, debug=False):
    import concourse.bacc as bacc
    import concourse.mybir as mybir
    import concourse.tile as tile

    has_bqv, has_bu, has_bo, has_affine = flags
    f32 = mybir.dt.float32
    bf16 = mybir.dt.bfloat16

    nc = bacc.Bacc("TRN2", target_bir_lowering=False, debug=False,
                   num_devices=N_CORES)

    t = {}
    t["xT"] = nc.dram_tensor("xT", [E, TL], bf16, kind="ExternalInput").ap()
    t["wqk"] = nc.dram_tensor("wqk", [E, H_DIM], bf16, kind="ExternalInput").ap()
    t["wv"] = nc.dram_tensor("wv", [E, H_DIM], bf16, kind="ExternalInput").ap()
    t["wu"] = nc.dram_tensor("wu", [E, H_DIM], bf16, kind="ExternalInput").ap()
    t["wo"] = nc.dram_tensor("wo", [H_DIM, E], bf16, kind="ExternalInput").ap()
    if has_bqv:
        t["bqk_r"] = nc.dram_tensor("bqk_r", [1, H_DIM], bf16,
                                    kind="ExternalInput").ap()
        t["bv_r"] = nc.dram_tensor("bv_r", [1, H_DIM], bf16,
                                   kind="ExternalInput").ap()
    if has_bo:
        t["bo_r"] = nc.dram_tensor("bo_r", [1, E], bf16,
                                   kind="ExternalInput").ap()
    if has_bu:
        t["bu_c"] = nc.dram_tensor("bu_c", [128, JT], f32,
                                   kind="ExternalInput").ap()
    if has_affine:
        t["g_c"] = nc.dram_tensor("g_c", [128, JT], f32,
                                  kind="ExternalInput").ap()
        t["b_c"] = nc.dram_tensor("b_c", [128, JT], f32,
                                  kind="ExternalInput").ap()
    t["y"] = nc.dram_tensor("y", [TL, E], bf16, kind="ExternalOutput").ap()

    dbg = None
    if debug:
        dbg = {
            "q": nc.dram_tensor("dbg_q", [TL, H_DIM], bf16,
                                kind="ExternalOutput").ap(),
            "uT": nc.dram_tensor("dbg_uT", [H_DIM, TL], bf16,
                                 kind="ExternalOutput").ap(),
            "kvout": nc.dram_tensor("dbg_kvout", [HEADS * B * 128, DH], f32,
                                    kind="ExternalOutput").ap(),
            "att": nc.dram_tensor("dbg_att", [B * 128, HEADS * NL], bf16,
                                  kind="ExternalOutput").ap(),
            "zT": nc.dram_tensor("dbg_zT", [B * 128, JT * NL], bf16,
                                 kind="ExternalOutput").ap(),
        }
    with tile.TileContext(nc) as tc:
        _body(nc, tc, tile, mybir, f32, bf16, t, flags, dbg)
    nc.compile()
    return nc


def _body(nc, tc, tile, mybir, f32, bf16, t, flags, dbg=None):
    Act = mybir.ActivationFunctionType
    Alu = mybir.AluOpType
    has_bqv, has_bu, has_bo, has_affine = flags

    with (
        tc.tile_pool(name="consts", bufs=1) as consts,
        tc.tile_pool(name="p3small", bufs=1) as p3small,
        tc.tile_pool(name="dram", bufs=1, space="DRAM") as dram,
    ):
        ones_col = consts.tile([128, 1], bf16)
        nc.vector.memset(ones_col, 1.0)
        eps_sb = consts.tile([1, 1], f32)
        nc.vector.memset(eps_sb, LN_EPS)
        if has_bqv or has_bo:
            ones_bf = consts.tile([1, 128], bf16)
            nc.vector.memset(ones_bf, 1.0)
        if has_bqv:
            bqk_sb = consts.tile([1, H_DIM], bf16)
            nc.sync.dma_start(bqk_sb[:], t["bqk_r"][:])
            bv_sb = consts.tile([1, H_DIM], bf16)
            nc.sync.dma_start(bv_sb[:], t["bv_r"][:])
        if has_bo:
            bo_sb = consts.tile([1, E], bf16)
            nc.sync.dma_start(bo_sb[:], t["bo_r"][:])
        if has_bu:
            bu_sb = consts.tile([128, JT], f32)
            nc.sync.dma_start(bu_sb[:], t["bu_c"][:])
        if has_affine:
            g_sb = consts.tile([128, JT], f32)
            nc.sync.dma_start(g_sb[:], t["g_c"][:])
            b_sb = consts.tile([128, JT], f32)
            nc.sync.dma_start(b_sb[:], t["b_c"][:])

        q_dram = dram.tile([TL, H_DIM], bf16)     # token-major q spill
        uT_dram = dram.tile([H_DIM, TL], bf16)
        # kv collective chunks: one per head-quarter, rows (hl*B + b)*128 + d
        cc_in = []
        cc_out = []
        for jq in range(NJQ):
            cc_in_t = dram.tile([4 * B * 128, DH], f32, name=f"cc_in{jq}")
            cc_in.append(cc_in_t)
            cc_out_t = dram.tile([4 * B * 128, DH], f32,
                                 addr_space="Shared", name=f"cc_out{jq}")
            cc_out.append(cc_out_t)

        # small early pools: their SBUF is disjoint from xt/w1, so the
        # phase-3 qb/kv loads can prefetch during phase 2 (no WAR on xt)
        kvf_ctx = tc.tile_pool(name="kvf", bufs=1)
        kvf = kvf_ctx.__enter__()
        qb_ctx = tc.tile_pool(name="qbp", bufs=1)
        qbp = qb_ctx.__enter__()

        with tc.tile_pool(name="xt_pool", bufs=1) as xt_pool:
            xt = xt_pool.tile([128, ET, TL], bf16)   # 8 MB resident ph1-2

            # w2 spans phases 1+2 with SBUF addresses disjoint from w1/st1,
            # so the wu prefetch overlaps phase 1 (no phase-boundary stall)
            w2_ctx = tc.tile_pool(name="w2", bufs=1)
            w2 = w2_ctx.__enter__()

            # ---------------- phase 1: q/v projections + partial kv --------
            with (
                tc.tile_pool(name="w1", bufs=1) as w1,
                tc.tile_pool(name="st1", bufs=1) as st1,
                tc.tile_pool(name="ps_proj", bufs=1, space="PSUM") as psp,
                tc.tile_pool(name="ps_kv", bufs=1, space="PSUM") as pskv,
            ):
                # phase-1 weights first in the DMA queue so the PE can start
                # ~8us in instead of waiting behind the 8 MB xT load
                wq_sb = {}
                wv_sb = {}
                wq_sb[0] = w1.tile([128, ET, 512], bf16, tag="wq", bufs=2,
                                   name="wq0")
                nc.sync.dma_start(
                    wq_sb[0][:],
                    t["wqk"][:, 0:512].rearrange("(t p) j -> p t j", p=128))
                wv_sb[0] = w1.tile([128, ET, 512], bf16, tag="wv", bufs=2,
                                   name="wv0")
                nc.sync.dma_start(
                    wv_sb[0][:],
                    t["wv"][:, 0:512].rearrange("(t p) j -> p t j", p=128))

                for tt in range(ET):
                    nc.sync.dma_start(
                        xt[:, tt],
                        t["xT"].rearrange("(t p) n -> t p n", p=128)[tt])

                wu_pre = []
                for jqu in range(2):
                    wu_q = w2.tile([128, ET, 512], bf16, tag="wu", bufs=2)
                    nc.scalar.dma_start(
                        wu_q[:],
                        t["wu"][:, jqu * 512:(jqu + 1) * 512]
                        .rearrange("(t p) j -> p t j", p=128))
                    wu_pre.append(wu_q)

                for jq in range(NJQ):
                    if jq > 0:
                        wq_sb[jq] = w1.tile([128, ET, 512], bf16, tag="wq",
                                            bufs=2, name=f"wq{jq}")
                        nc.sync.dma_start(
                            wq_sb[jq][:],
                            t["wqk"][:, jq * 512:(jq + 1) * 512]
                            .rearrange("(t p) j -> p t j", p=128))
                        wv_sb[jq] = w1.tile([128, ET, 512], bf16, tag="wv",
                                            bufs=2, name=f"wv{jq}")
                        nc.sync.dma_start(
                            wv_sb[jq][:],
                            t["wv"][:, jq * 512:(jq + 1) * 512]
                            .rearrange("(t p) j -> p t j", p=128))
                    for b in range(B):
                        q_tiles, v_tiles = [], []
                        for tk in range(TOK_B):
                            tok0 = b * NL + tk * 128
                            q_ps = psp.tile([128, 512], f32, tag="qps", bufs=2)
                            v_ps = psp.tile([128, 512], f32, tag="vps", bufs=2)
                            for tt in range(ET):
                                lhs = xt[:, tt, tok0:tok0 + 128]
                                nc.tensor.matmul(q_ps[:], lhs,
                                                 wq_sb[jq][:, tt],
                                                 start=(tt == 0), stop=False)
                                nc.tensor.matmul(
                                    v_ps[:], lhs, wv_sb[jq][:, tt],
                                    start=(tt == 0),
                                    stop=(not has_bqv and tt == ET - 1))
                            if has_bqv:
                                nc.tensor.matmul(
                                    q_ps[:], ones_bf[:],
                                    bqk_sb[:, jq * 512:(jq + 1) * 512],
                                    start=False, stop=True)
                                nc.tensor.matmul(
                                    v_ps[:], ones_bf[:],
                                    bv_sb[:, jq * 512:(jq + 1) * 512],
                                    start=False, stop=True)

                            # elu(q) = (max(q,0) - 1) + exp(min(q, 0))
                            tmin = st1.tile([128, 512], f32, tag="tmin",
                                            bufs=2)
                            nc.vector.tensor_scalar_min(tmin[:], q_ps[:], 0.0)
                            texp = st1.tile([128, 512], f32, tag="texp",
                                            bufs=2)
                            nc.scalar.activation(texp[:], tmin[:], Act.Exp)
                            trelu = st1.tile([128, 512], f32, tag="trelu",
                                             bufs=2)
                            nc.vector.tensor_scalar(trelu[:], q_ps[:], 0.0,
                                                    -1.0, Alu.max, Alu.add)
                            q_bf = st1.tile([128, 512], bf16, tag="qbf",
                                            bufs=4)
                            nc.vector.tensor_add(q_bf[:], trelu[:], texp[:])
                            # silu(v) = 0.5*v*(1+tanh(v/2)) — keeps the exp
                            # table set loaded (tanh lives in it; Silu not)
                            vth = st1.tile([128, 512], f32, tag="vth", bufs=1)
                            nc.scalar.activation(vth[:], v_ps[:], Act.Tanh,
                                                 scale=0.5)
                            vsg = st1.tile([128, 512], f32, tag="vsg", bufs=1)
                            nc.vector.tensor_scalar(vsg[:], vth[:], 0.5, 0.5,
                                                    Alu.mult, Alu.add)
                            v_bf = st1.tile([128, 512], bf16, tag="vbf",
                                            bufs=4)
                            nc.vector.tensor_mul(v_bf[:], v_ps[:], vsg[:])
                            q_tiles.append(q_bf)
                            v_tiles.append(v_bf)

                            # spill q token-major; phase 3 reloads it through
                            # the XBAR transposing DMA
                            nc.sync.dma_start(
                                q_dram[tok0:tok0 + 128,
                                       jq * 512:(jq + 1) * 512],
                                q_bf[:])

                        # per-head contiguous kv accumulation: each head owns
                        # a whole PSUM bank (start=True clears the full bank,
                        # so accumulation groups must not share banks)
                        kv_sb = st1.tile([128, 4, DH], f32, tag="kvsb",
                                         bufs=1)
                        for h in range(4):
                            kv_ps = pskv.tile([128, DH], f32, tag="kv",
                                              bufs=2)
                            for tk in range(TOK_B):
                                nc.tensor.matmul(
                                    kv_ps[:],
                                    q_tiles[tk][:, h * 128:(h + 1) * 128],
                                    v_tiles[tk][:, h * 128:(h + 1) * 128],
                                    start=(tk == 0), stop=(tk == TOK_B - 1))
                            nc.vector.tensor_copy(kv_sb[:, h], kv_ps[:])
                        nc.sync.dma_start(
                            cc_in[jq].rearrange("(h b p) e -> b p h e",
                                                h=4, b=B)[b],
                            kv_sb[:])

                    # AllReduce this head-quarter as soon as its kv is out;
                    # chunks overlap the phase-1 tail + the u-projection
                    nc.gpsimd.collective_compute(
                        "AllReduce", mybir.AluOpType.add,
                        replica_groups=[list(range(N_CORES))],
                        ins=[cc_in[jq].opt()], outs=[cc_out[jq].opt()])

            # ---------------- phase 2: uT projection (overlaps AR) ---------
            with (
                tc.tile_pool(name="ps_u", bufs=1, space="PSUM") as psu,
            ):
                for jqu in range(NJQ):
                    if jqu < 2:
                        wu_q = wu_pre[jqu]
                    else:
                        wu_q = w2.tile([128, ET, 512], bf16, tag="wu", bufs=2)
                        nc.scalar.dma_start(
                            wu_q[:],
                            t["wu"][:, jqu * 512:(jqu + 1) * 512]
                            .rearrange("(t p) j -> p t j", p=128))
                    for jl in range(4):
                        jt = jqu * 4 + jl
                        u_ps = psu.tile([128, 4, 512], f32, tag="ups", bufs=2)
                        for tt in range(ET):
                            for c in range(4):
                                nc.tensor.matmul(
                                    u_ps[:, c],
                                    wu_q[:, tt, jl * 128:(jl + 1) * 128],
                                    xt[:, tt, c * 512:(c + 1) * 512],
                                    start=(tt == 0), stop=(tt == ET - 1))
                        u_st = w2.tile([128, TL], bf16, tag="ust", bufs=1)
                        ubias = bu_sb[:, jt:jt + 1] if has_bu else 0.0
                        for c in range(4):
                            nc.scalar.activation(
                                u_st[:, c * 512:(c + 1) * 512], u_ps[:, c],
                                Act.Silu, bias=ubias)
                        nc.scalar.dma_start(
                            uT_dram[jt * 128:(jt + 1) * 128, :], u_st[:])
            w2_ctx.__exit__(None, None, None)

        if dbg is not None:
            nc.sync.dma_start(dbg["q"][:], q_dram[:])
            nc.sync.dma_start(dbg["uT"][:], uT_dram[:])
            for jq in range(NJQ):
                nc.sync.dma_start(
                    dbg["kvout"][jq * 4 * B * 128:(jq + 1) * 4 * B * 128, :],
                    cc_out[jq][:])

        # ------------- phase 3+4: attention, layernorm, output proj --------
        with (
            tc.tile_pool(name="wo_pool", bufs=1) as wo_pool,
            tc.tile_pool(name="st3", bufs=1) as st3,
            tc.tile_pool(name="utp", bufs=1) as utp,
            tc.tile_pool(name="ps_att", bufs=1, space="PSUM") as psa,
            tc.tile_pool(name="ps_sm", bufs=1, space="PSUM") as pssm,
            tc.tile_pool(name="ps_y", bufs=1, space="PSUM") as psy,
        ):
            wo_sb = wo_pool.tile([128, JT, E], bf16)     # 8 MB resident
            for ct in range(JT):
                nc.scalar.dma_start(
                    wo_sb[:, ct],
                    t["wo"].rearrange("(t p) e -> t p e", p=128)[ct])

            att_tiles = {}
            sq_tiles = {}
            stat_tiles = {}

            def att_block(b):
                kv_bf = kvf.tile([128, HEADS, DH], bf16, tag="kvbf", bufs=2)
                for jq in range(NJQ):
                    nc.gpsimd.dma_start(
                        kv_bf[:, jq * 4:(jq + 1) * 4],
                        cc_out[jq].rearrange("(h b p) e -> b p h e",
                                             h=4, b=B)[b])
                att = st3.tile([128, HEADS, NL], bf16, tag="att", bufs=2)
                att_tiles[b] = att
                sq = st3.tile([128, HEADS, NL], bf16, tag="sq", bufs=1)
                sq_tiles[b] = sq
                for h in range(HEADS):
                    qb = qbp.tile([128, NL], bf16, tag="qb", bufs=3)
                    nc.sync.dma_start_transpose(
                        qb[:],
                        q_dram[b * NL:(b + 1) * NL, h * 128:(h + 1) * 128])
                    # att_ps has two readers (DVE copy + ACT square); bufs=3
                    # keeps the matmul stream ahead of the reader latency
                    att_ps = psa.tile([128, NL], f32, tag="attps", bufs=3)
                    nc.tensor.matmul(att_ps[:], kv_bf[:, h], qb[:],
                                     start=True, stop=True)
                    nc.vector.tensor_copy(att[:, h], att_ps[:])
                    nc.scalar.activation(sq[:, h], att[:, h], Act.Square)

            def stats_block(b):
                att = att_tiles[b]
                sq = sq_tiles.pop(b)
                # LN stats over channels via ones-matmuls (bf16 operands)
                sum_ps = pssm.tile([1, NL], f32, tag="sum", bufs=1)
                ssq_ps = pssm.tile([1, NL], f32, tag="ssq", bufs=1)
                for h in range(HEADS):
                    nc.tensor.matmul(sum_ps[:], ones_col[:], att[:, h],
                                     start=(h == 0), stop=(h == HEADS - 1))
                for h in range(HEADS):
                    nc.tensor.matmul(ssq_ps[:], ones_col[:], sq[:, h],
                                     start=(h == 0), stop=(h == HEADS - 1))
                stat_tiles[b] = (sum_ps, ssq_ps)

            def chain_block(b):
                # small [1,NL] tiles cost full per-partition column space, so
                # the chain recycles three tags by value lifetime
                sum_ps, ssq_ps = stat_tiles.pop(b)
                mean = st3.tile([1, NL], f32, tag="cA", bufs=1, name="mean")
                nc.vector.tensor_scalar_mul(mean[:], sum_ps[:], 1.0 / H_DIM)
                m2 = st3.tile([1, NL], f32, tag="cB", bufs=1, name="m2")
                nc.scalar.activation(m2[:], mean[:], Act.Square)
                var = st3.tile([1, NL], f32, tag="cC", bufs=1, name="var")
                nc.vector.scalar_tensor_tensor(
                    var[:], ssq_ps[:], 1.0 / H_DIM, m2[:],
                    Alu.mult, Alu.subtract)
                lnv = st3.tile([1, NL], f32, tag="cB", bufs=1, name="lnv")
                nc.scalar.activation(lnv[:], var[:], Act.Ln, bias=eps_sb[:])
                rstd = st3.tile([1, NL], f32, tag="cC", bufs=1, name="rstd")
                nc.scalar.activation(rstd[:], lnv[:], Act.Exp, scale=-0.5)
                mr = st3.tile([1, NL], f32, tag="cB", bufs=1, name="mr")
                nc.vector.tensor_mul(mr[:], mean[:], rstd[:])
                rstd_b16 = st3.tile([1, NL], bf16, tag="c16a", bufs=1,
                                    name="rstd_b16")
                nc.vector.tensor_copy(rstd_b16[:], rstd[:])
                mr_b16 = st3.tile([1, NL], bf16, tag="c16b", bufs=1,
                                  name="mr_b16")
                nc.vector.tensor_copy(mr_b16[:], mr[:])
                rstd_bc = st3.tile([128, NL], bf16, tag="rstdbc", bufs=2)
                nc.gpsimd.partition_broadcast(rstd_bc[:], rstd_b16[:])
                mr_bc = st3.tile([128, NL], bf16, tag="mrbc", bufs=2)
                nc.gpsimd.partition_broadcast(mr_bc[:], mr_b16[:])
                return rstd_bc, mr_bc

            ut_loaded = {}

            def load_ut(b):
                uT_b = utp.tile([128, JT, NL], bf16, tag="utb", bufs=1)
                nc.sync.dma_start(
                    uT_b[:],
                    uT_dram[:, b * NL:(b + 1) * NL]
                    .rearrange("(jt p) n -> p jt n", p=128))
                ut_loaded[b] = uT_b

            zT_tiles = {}

            def zT_block(b, bc):
                rstd_bc, mr_bc = bc
                att = att_tiles.pop(b)
                uT_b = ut_loaded.pop(b)
                zT = st3.tile([128, JT, NL], bf16, tag="zT", bufs=2)
                zT_tiles[b] = zT
                # (att*rstd - mean*rstd) [*g+b] * u — bf16 on DVE
                for qd in range(4):
                    s1 = st3.tile([128, 4, NL], bf16, tag="s1", bufs=1)
                    nc.vector.tensor_mul(
                        s1[:], att[:, 4 * qd:4 * (qd + 1)],
                        rstd_bc[:, None, :].broadcast_to([128, 4, NL]))
                    s2 = st3.tile([128, 4, NL], bf16, tag="s2", bufs=1)
                    nc.vector.tensor_sub(
                        s2[:], s1[:],
                        mr_bc[:, None, :].broadcast_to([128, 4, NL]))
                    if has_affine:
                        s3 = st3.tile([128, 4, NL], bf16, tag="s3", bufs=1)
                        for jl in range(4):
                            jt = 4 * qd + jl
                            nc.vector.tensor_scalar(
                                s3[:, jl], s2[:, jl], g_sb[:, jt:jt + 1],
                                b_sb[:, jt:jt + 1], Alu.mult, Alu.add)
                    else:
                        s3 = s2
                    nc.vector.tensor_mul(zT[:, 4 * qd:4 * (qd + 1)], s3[:],
                                         uT_b[:, 4 * qd:4 * (qd + 1)])
                if dbg is not None:
                    nc.sync.dma_start(dbg["zT"][b * 128:(b + 1) * 128, :],
                                      zT[:])

            def yproj_block(b):
                zT = zT_tiles.pop(b)
                # y = zT.T @ Wo + bo; e-quarter PSUM tiles (1 bank, bufs=3)
                # so the evacuation copies overlap the next quarter's matmuls
                for tsl in range(TOK_B):
                    ybuf = st3.tile([128, E], bf16, tag="ybuf", bufs=2)
                    for e4 in range(4):
                        e0 = e4 * 512
                        y_ps = psy.tile([128, 512], f32, tag="yps", bufs=3)
                        for ct in range(JT):
                            zslice = zT[:, ct, tsl * 128:(tsl + 1) * 128]
                            nc.tensor.matmul(
                                y_ps[:], zslice,
                                wo_sb[:, ct, e0:e0 + 512],
                                start=(ct == 0),
                                stop=(not has_bo and ct == JT - 1))
                        if has_bo:
                            nc.tensor.matmul(
                                y_ps[:], ones_bf[:],
                                bo_sb[:, e0:e0 + 512],
                                start=False, stop=True)
                        nc.scalar.copy(ybuf[:, e0:e0 + 512], y_ps[:])
                    nc.sync.dma_start(
                        t["y"][b * NL + tsl * 128: b * NL + (tsl + 1) * 128,
                               :],
                        ybuf[:])

            # software-pipelined schedule: PE stream is
            #   att0 att1 stats0 att2 stats1 att3 stats2 yproj0 stats3 ...
            # while each batch's LN chain (ACT/DVE/GpSimd) resolves under the
            # next batches' matmuls.
            # PE stream: att0 att1 stats0 att2 stats1 att3 stats2 yproj0 ...
            # each batch's LN chain (ACT/DVE) + zT (GpSimd) resolves under
            # the next batches' matmuls; zT(b) is emitted right after
            # chain(b) so att buffers recycle at bufs=3.
            load_ut(0)
            att_block(0)
            load_ut(1)
            att_block(1)
            stats_block(0)
            bc0 = chain_block(0)
            zT_block(0, bc0)
            att_block(2)
            load_ut(2)
            stats_block(1)
            bc1 = chain_block(1)
            zT_block(1, bc1)
            att_block(3)
            load_ut(3)
            stats_block(2)
            bc2 = chain_block(2)
            yproj_block(0)
            zT_block(2, bc2)
            stats_block(3)
            bc3 = chain_block(3)
            yproj_block(1)
            zT_block(3, bc3)
            yproj_block(2)
            yproj_block(3)

            if dbg is not None:
                for b in range(B):
                    att = st3.tile([128, HEADS, NL], bf16, tag="attd", bufs=1)
                    kv_f = kvf.tile([128, HEADS, DH], f32, tag="kvf", bufs=1)
                    for jq in range(NJQ):
                        nc.sync.dma_start(
                            kv_f[:, jq * 4:(jq + 1) * 4],
                            cc_out[jq].rearrange("(h b p) e -> b p h e",
                                                 h=4, b=B)[b])
                    kv_bf = kvf.tile([128, HEADS, DH], bf16, tag="kvbf",
                                     bufs=2)
                    nc.vector.tensor_copy(kv_bf[:], kv_f[:])
                    for h in range(HEADS):
                        qb = qbp.tile([128, NL], bf16, tag="qb", bufs=6)
                        nc.sync.dma_start_transpose(
                            qb[:],
                            q_dram[b * NL:(b + 1) * NL,
                                   h * 128:(h + 1) * 128])
                        att_ps = psa.tile([128, NL], f32, tag="attps", bufs=2)
                        nc.tensor.matmul(att_ps[:], kv_bf[:, h], qb[:],
                                         start=True, stop=True)
                        nc.scalar.copy(att[:, h], att_ps[:])
                    nc.sync.dma_start(dbg["att"][b * 128:(b + 1) * 128, :],
                                      att[:])

        qb_ctx.__exit__(None, None, None)
        kvf_ctx.__exit__(None, None, None)



def _get_nc(flags, debug=False):
    key = (flags, debug)
    if key not in _BUILT:
        _BUILT[key] = _build(flags, debug)
    return _BUILT[key]


def make_in_maps(x, Wqk, bqk, Wv, bv, Wu, bu, Wo, bo, ln_g, ln_b):
    bf16 = ml_dtypes.bfloat16
    f32 = np.float32
    x = np.asarray(x)
    flags = (
        bool(np.any(bqk) or np.any(bv)),
        bool(np.any(bu)),
        bool(np.any(bo)),
        bool(np.any(np.asarray(ln_g) != 1.0) or np.any(ln_b)),
    )
    shared = {
        "wqk": np.asarray(Wqk, f32).astype(bf16),
        "wv": np.asarray(Wv, f32).astype(bf16),
        "wu": np.asarray(Wu, f32).astype(bf16),
        "wo": np.asarray(Wo, f32).astype(bf16),
    }
    if flags[0]:
        shared["bqk_r"] = np.asarray(bqk, f32).astype(bf16).reshape(1, H_DIM)
        shared["bv_r"] = np.asarray(bv, f32).astype(bf16).reshape(1, H_DIM)
    if flags[1]:
        shared["bu_c"] = np.ascontiguousarray(
            np.asarray(bu, f32).reshape(JT, 128).T)
    if flags[2]:
        shared["bo_r"] = np.asarray(bo, f32).astype(bf16).reshape(1, E)
    if flags[3]:
        shared["g_c"] = np.ascontiguousarray(
            np.asarray(ln_g, f32).reshape(JT, 128).T)
        shared["b_c"] = np.ascontiguousarray(
            np.asarray(ln_b, f32).reshape(JT, 128).T)
    in_maps = []
    for c in range(N_CORES):
        xc = np.ascontiguousarray(
            x[:, c * NL:(c + 1) * NL, :].reshape(TL, E).T).astype(bf16)
        in_maps.append({"xT": xc, **shared})
    return flags, in_maps


def kernel(x, Wqk, bqk, Wv, bv, Wu, bu, Wo, bo, ln_g, ln_b, **_unused):
    from concourse.bass_utils import run_bass_kernel_spmd

    flags, in_maps = make_in_maps(x, Wqk, bqk, Wv, bv, Wu, bu, Wo, bo,
                                  ln_g, ln_b)
    nc = _get_nc(flags)
    res = run_bass_kernel_spmd(nc, in_maps, core_ids=list(range(N_CORES)))

    y = np.empty((B, N, E), np.float32)
    for c in range(N_CORES):
        y[:, c * NL:(c + 1) * NL, :] = (
            res.results[c]["y"].astype(np.float32).reshape(B, NL, E))
    return y
